# revision 51
# baseline (speedup 1.0000x reference)
"""Block-causal (anti-causal: key-block >= query-block) multi-head attention
for Trainium2, run SPMD on 8 NeuronCores.

Problem (hardcoded): B=2, T=8, N=256 (L=2048), D=768, H=12, HD=64.
reference:
    qkv = x @ qkv_w.T + qkv_b ; split into q,k,v heads
    s   = (q @ k.T) / 8 ; mask: query in block ti attends keys in blocks tj >= ti
    p   = softmax(s) ; y = p @ v ; out = y @ proj_w.T + proj_b

Sharding: data-parallel over B (2) x tensor-parallel over heads (4 groups of
3 heads) = 8 cores. Each core computes, for its (batch, head-group):
  - Q^T,K^T = Wqk @ x^T   (bf16 matmuls; bias folded into the PSUM->SBUF
              copy on the scalar engine: Identity activation + bias AP)
  - Vn      = x-chunk.T @ Wv-stream: V in NATURAL layout [keys, 3*(64+1)]
              per 128-key chunk (xT chunk stationary, V weights moving).
              The per-head ones column (softmax denominator) is a zero
              weight column plus 1.0 in the broadcast bias added on DVE.
  - S^T     = K^T.T-chunks vs Q^T   (keys on partitions, queries on free dim)
  - P~      = exp(0.125 * S^T)      (no max-subtraction; logits are tiny)
  - U^T     = Vn.T @ P~              (ones column gives denominator row)
  - O^T     = U^T * (1/den) broadcast (recip on DVE, partition-broadcast on
              GpSimd, multiply on DVE)
  - Z^T    += Wproj-slice @ O^T      (partial projection output, bf16)
Host sums the 4 head-group partials per batch and adds proj_b.

Schedule: input DMAs are split dc-wise across the sync and gpsimd queues
(~145 GB/s each) in matched priority order.  K/Q chains run first
(DMA-paced); the qk2 and natural-V chains are then interleaved with
pre-emitted S+exp key-chunk pairs.  EVERY attention group's S pairs are
emitted ~2 groups ahead of its PV matmuls (uniform lookahead, throttled by
the 2-buffer S-tile PSUM ring): exp starts ~20us in, no group's first PV
waits on its own exp, and no exp work remains in the tail.  Key chunks
are reordered so the first PV matmul of each group covers the full
512-query PSUM bank with start=True (pending-zero is bank-granular).  The
projection for quarter qq is emitted during the attention of quarter qq+1
so the output DMA overlaps compute; its casts run on DVE early and
alternate scalar/DVE for the last two quarters (after exp drains).  A
burst of K=128 dummy matmuls precedes the last projection: it keeps the
tensor engine's utilization-driven clock ramped across the final
normalization-chain stall (measured ~20us whole-kernel effect, A-B-A
verified; do NOT emit such dummies while input DMAs stream — their SBUF
reads throttle the DMA engines).
"""

import functools

import ml_dtypes
import numpy as np

import concourse.bass as bass
import concourse.bacc as bacc_mod
import concourse.mybir as mybir
import concourse.tile as tile
from concourse.bass import ts

F32 = mybir.dt.float32
BF16 = mybir.dt.bfloat16

B, T, N, D = 2, 8, 256, 768
H, HD = 12, 64
L = T * N          # 2048
HPC = 3            # heads per core
NKC = L // 128     # 16 key chunks of 128
NDC = D // 128     # 6 contraction chunks
SCALE = 1.0 / 8.0
VW = 3 * 65        # natural-V row width: [v_h0 | 1] [v_h1 | 1] [v_h2 | 1]


def group_pairs(qq):
    """Key-chunk pairs for one (head, qq) group; masked pair last."""
    kcs = list(range(4 * qq + 2, 16)) + [4 * qq, 4 * qq + 1]
    return [(kcs[i], kcs[i + 1]) for i in range(0, len(kcs), 2)]


def build_nc():
    nc = bacc_mod.Bacc()

    xT_d = nc.declare_dram_parameter("xT", [D, L], BF16, isOutput=False)
    # weights stored partition-major on the host: contiguous multi-KB DMA lines
    wqkT_d = nc.declare_dram_parameter("wqkT", [128, NDC * 384], BF16, isOutput=False)
    wvT3_d = nc.declare_dram_parameter("wvT3", [128, NDC * VW], BF16, isOutput=False)
    bqk_d = nc.declare_dram_parameter("bqk", [128, 4], F32, isOutput=False)
    bvb_d = nc.declare_dram_parameter("bvb", [128, VW], F32, isOutput=False)
    wprojT_d = nc.declare_dram_parameter("wprojT", [128, 1536], BF16, isOutput=False)
    zT_d = nc.declare_dram_parameter("zT", [D, L], BF16, isOutput=True)

    with tile.TileContext(nc) as tc:
        with (
            tc.tile_pool(name="persist", bufs=1) as pp,
            tc.tile_pool(name="ptile", bufs=26) as ppool,
            tc.tile_pool(name="zbuf", bufs=3) as zpool,
            tc.tile_pool(name="psum_st", bufs=2, space="PSUM") as pst,
            tc.tile_pool(name="psum_ot", bufs=2, space="PSUM") as pot,
            tc.tile_pool(name="psum_mc", bufs=2, space="PSUM") as pmc,
        ):
            # ---- persistent SBUF tensors ----
            wqkT = pp.tile([128, NDC, 384], BF16, tag="wqkT")
            wvT3 = pp.tile([128, NDC, VW], BF16, tag="wvT3")
            bqk = pp.tile([128, 4], F32, tag="bqk")
            bvb = pp.tile([128, VW], F32, tag="bvb")
            wprojT = pp.tile([128, 1536], BF16, tag="wprojT")
            qt = pp.tile([128, L], BF16, tag="qt")      # [q_h0 | q_h1]
            kt = pp.tile([128, L], BF16, tag="kt")      # [k_h0 | k_h1]
            qk2 = pp.tile([128, L], BF16, tag="qk2")    # [q_h2 | k_h2]
            kt2 = pp.tile([64, L], BF16, tag="kt2")     # k_h2 re-based to part 0
            vn = pp.tile([128, NKC, VW], BF16, tag="vn")
            otp = pp.tile([128, L], BF16, tag="otp")    # [o_h0 | o_h1]
            ots = pp.tile([64, L], BF16, tag="ots")     # [o_h2]
            bcast = pp.tile([64, 512], F32, tag="bcast")
            den = pp.tile([1, 512], F32, tag="den")

            qt_src = [qt[0:64, :], qt[64:128, :], qk2[0:64, :]]
            kt_src = [kt[0:64, :], kt[64:128, :], kt2[0:64, :]]
            ot_dst = [otp[0:64, :], otp[64:128, :], ots[0:64, :]]
            scratch = pp.tile([128, 512], BF16, tag="scratch")
            nc.gpsimd.memset(scratch[:], 0.0)

            def pe_warm(n):
                """K=128 dummy matmuls: keep the PE's utilization-driven
                clock ramped across a known stall.  Only safe when no input
                DMA is streaming (the K=128 SBUF reads throttle DMA)."""
                for _ in range(n):
                    ds = pmc.tile([128, 512], F32, tag="qs")
                    nc.tensor.matmul(
                        ds[:], scratch[:, 0:128], scratch[:],
                        start=True, stop=True,
                    )



            def emit_s_pair(h, qq, pi, a, b):
                """S matmuls + exp for one key-chunk pair; returns a PV job."""
                q_lo = qq * 512
                masked = a == 4 * qq
                seg = 256 if masked else 512
                st2 = pst.tile([128, 1024], F32, tag="st")
                nc.tensor.matmul(
                    st2[:, 0:seg],
                    kt_src[h][:, ts(a, 128)],
                    qt_src[h][:, q_lo : q_lo + seg],
                    start=True, stop=True,
                )
                nc.tensor.matmul(
                    st2[:, 512 : 512 + seg],
                    kt_src[h][:, ts(b, 128)],
                    qt_src[h][:, q_lo : q_lo + seg],
                    start=True, stop=True,
                )
                pt = ppool.tile([128, 1024], BF16, tag="pt")
                if masked:
                    for off in (0, 512):
                        nc.scalar.activation(
                            pt[:, off : off + 256],
                            st2[:, off : off + 256],
                            mybir.ActivationFunctionType.Exp,
                            scale=SCALE,
                        )
                else:
                    nc.scalar.activation(
                        pt[:, 0:1024],
                        st2[:, 0:1024],
                        mybir.ActivationFunctionType.Exp,
                        scale=SCALE,
                    )
                return (pi, a, b, pt, masked)

            def emit_pv(h, ot, job):
                pi, a, b, pt, masked = job
                va = vn[:, a, 65 * h : 65 * h + 65]
                vb = vn[:, b, 65 * h : 65 * h + 65]
                if not masked:
                    nc.tensor.matmul(
                        ot[0:65, 0:512], va, pt[:, 0:512],
                        start=(pi == 0), stop=False, skip_group_check=True,
                    )
                    nc.tensor.matmul(
                        ot[0:65, 0:512], vb, pt[:, 512:1024],
                        start=False, stop=False, skip_group_check=True,
                    )
                else:
                    nc.tensor.matmul(
                        ot[0:65, 0:256], va, pt[:, 0:256],
                        start=False, stop=False, skip_group_check=True,
                    )
                    nc.tensor.matmul(
                        ot[0:65, 0:256], vb, pt[:, 512:768],
                        start=False, stop=True, skip_group_check=True,
                    )

            # ---- input DMAs ----
            # Two hardware queues (sync + gpsimd) at ~145 GB/s each; every
            # transfer is split dc-wise across both, in matched priority
            # order, so each arrival (xT chunk, weight block) lands at the
            # full ~290 GB/s.  Host stores wqkT blocks K|Q|qk2.
            with tc.tile_pool(name="xT", bufs=1) as xp:
                xT = xp.tile([128, NDC, L], BF16, tag="xT")

                def x_dma(eng, nt, lo, hi):
                    eng.dma_start(
                        out=xT[:, lo:hi, ts(nt, 512)],
                        in_=xT_d[128 * lo : 128 * hi, ts(nt, 512)].rearrange(
                            "(dc p) w -> p dc w", p=128
                        ),
                    )

                def w_dma(eng, blk, out_cols, lo, hi):
                    eng.dma_start(
                        out=wqkT[:, lo:hi, out_cols[0] : out_cols[1]],
                        in_=wqkT_d[:, 768 * blk + 128 * lo : 768 * blk + 128 * hi]
                        .rearrange("p (dc w) -> p dc w", w=128),
                    )

                x_dma(nc.sync, 0, 0, 3)
                x_dma(nc.gpsimd, 0, 3, 6)
                nc.gpsimd.dma_start(out=bqk[:], in_=bqk_d[:, :])
                w_dma(nc.sync, 0, (128, 256), 0, 3)      # K block
                w_dma(nc.gpsimd, 0, (128, 256), 3, 6)
                w_dma(nc.sync, 1, (0, 128), 0, 3)        # Q block
                w_dma(nc.gpsimd, 1, (0, 128), 3, 6)
                for nt in range(1, 4):
                    x_dma(nc.sync, nt, 0, 3)
                    x_dma(nc.gpsimd, nt, 3, 6)
                w_dma(nc.sync, 2, (256, 384), 0, 3)      # qk2 block
                w_dma(nc.gpsimd, 2, (256, 384), 3, 6)
                nc.sync.dma_start(
                    out=wvT3[:, 0:3, :],
                    in_=wvT3_d[:, 0 : 3 * VW].rearrange("p (dc w) -> p dc w", w=VW),
                )
                nc.gpsimd.dma_start(
                    out=wvT3[:, 3:6, :],
                    in_=wvT3_d[:, 3 * VW :].rearrange("p (dc w) -> p dc w", w=VW),
                )
                nc.gpsimd.dma_start(out=bvb[:], in_=bvb_d[:, :])
                nc.sync.dma_start(out=wprojT[:, 0:768], in_=wprojT_d[:, 0:768])
                nc.gpsimd.dma_start(out=wprojT[:, 768:1536], in_=wprojT_d[:, 768:1536])
                nc.vector.memset(bcast[:], 1.0)
                # Pre-warm the exp table immediately: the scalar engine runs
                # exp-only from here on (all bias copies live on DVE).
                warm = zpool.tile([128, 32], F32, tag="warm")
                nc.vector.memset(warm[:], 0.0)
                nc.scalar.activation(
                    warm[:], warm[:], mybir.ActivationFunctionType.Exp
                )

                # ---- phase 1: qk chains + natural-V chains, interleaved with
                # pre-emitted S+exp pairs.  EVERY attention group's S pairs
                # are emitted ~2 groups ahead of its PV matmuls (uniform
                # lookahead): exp starts ~20us in, no group's first PV waits
                # on its own exp, and the tail has no exp left to run. ----
                groups_order = [(h, qq) for qq in range(4) for h in range(HPC)]
                pre_jobs = {hq: [] for hq in groups_order}
                s_slots = [
                    (gi, hq, pi, a, b)
                    for gi, hq in enumerate(groups_order)
                    for pi, (a, b) in enumerate(group_pairs(hq[1]))
                ]
                s_done = 0
                kt2_emitted = False

                def emit_next_s(ready_nt, max_gi, limit=1):
                    """Emit queued S pairs whose inputs have landed (kt is
                    written nt-progressively; h2 groups need the kt2 rebase
                    DMA emitted first), up to group index max_gi."""
                    nonlocal s_done
                    while s_done < len(s_slots) and limit > 0:
                        gi, hq, pi, a, b = s_slots[s_done]
                        if gi > max_gi:
                            return
                        if max(a, b) >= 4 * (ready_nt + 1):
                            return
                        if hq[0] == 2 and not kt2_emitted:
                            return
                        pre_jobs[hq].append(emit_s_pair(hq[0], hq[1], pi, a, b))
                        s_done += 1
                        limit -= 1

                def qk_chain(mc, dst, nt):
                    ps = pmc.tile([128, 512], F32, tag="qs")
                    for dc in range(NDC):
                        nc.tensor.matmul(
                            ps[:],
                            wqkT[:, dc, ts(mc, 128)],
                            xT[:, dc, ts(nt, 512)],
                            start=(dc == 0),
                            stop=(dc == NDC - 1),
                        )
                    nc.vector.tensor_scalar_add(
                        dst[:, ts(nt, 512)], ps[:], bqk[:, mc : mc + 1]
                    )

                qk_chain(1, kt, 0)
                qk_chain(0, qt, 0)
                for nt in range(1, 4):
                    qk_chain(1, kt, nt)       # keys: S pairs consume these
                    emit_next_s(nt - 1, 2, limit=2)
                    qk_chain(0, qt, nt)
                    emit_next_s(nt - 1, 2, limit=2)
                for nt in range(4):
                    # qk2 chain (bias on DVE like the rest)
                    ps = pmc.tile([128, 512], F32, tag="qs")
                    for dc in range(NDC):
                        nc.tensor.matmul(
                            ps[:],
                            wqkT[:, dc, 256:384],
                            xT[:, dc, ts(nt, 512)],
                            start=(dc == 0),
                            stop=(dc == NDC - 1),
                        )
                    nc.vector.tensor_scalar_add(
                        qk2[:, ts(nt, 512)], ps[:], bqk[:, 2:3]
                    )
                    if nt == 3:
                        # k_h2 re-base: partitions 64:128 -> 0:64
                        nc.gpsimd.dma_start(out=kt2[0:64, :], in_=qk2[64:128, :])
                        kt2_emitted = True
                    for kc in range(4 * nt, 4 * nt + 4):
                        vp = pot.tile([128, 256], F32, tag="ot")
                        for dc in range(NDC):
                            nc.tensor.matmul(
                                vp[:, 0:VW],
                                xT[:, dc, ts(kc, 128)],
                                wvT3[:, dc, :],
                                start=(dc == 0),
                                stop=(dc == NDC - 1),
                            )
                        nc.vector.tensor_tensor(
                            out=vn[:, kc, :],
                            in0=vp[:, 0:VW],
                            in1=bvb[:],
                            op=mybir.AluOpType.add,
                        )
                        emit_next_s(3, 2)
                emit_next_s(3, 2, limit=99)  # drain groups 0..2 leftovers

            # ---- attention + interleaved projection ----
            def attn_group(h, qq, lookahead_gi):
                ot = pot.tile([128, 512], F32, tag="ot")
                jobs = pre_jobs.pop((h, qq))
                assert len(jobs) == len(group_pairs(qq))
                for job in jobs:
                    emit_pv(h, ot, job)
                    # one lookahead S pair (group +2) per PV slot
                    emit_next_s(3, lookahead_gi, limit=1)
                # normalize: inv = 1/den, broadcast across 64 partitions.
                # den goes via SBUF: custom-DVE reciprocal reading PSUM
                # returns garbage on HW (verified; sim does not model it).
                q_lo = qq * 512
                nc.vector.tensor_copy(den[0:1, :], ot[64:65, 0:512])
                nc.vector.reciprocal_approx_fast(bcast[0:1, :], den[0:1, :])
                nc.gpsimd.partition_broadcast(bcast[0:64, :], bcast[0:1, :])
                nc.vector.tensor_tensor(
                    out=ot_dst[h][:, q_lo : q_lo + 512],
                    in0=ot[0:64, 0:512],
                    in1=bcast[0:64, :],
                    op=mybir.AluOpType.mult,
                )

            def proj(qq, casts_on_scalar=False, use_pst=False):
                for mc in range(NDC):
                    # alternate PSUM pools: 4 rotating banks hide cast latency
                    # (the tail projection borrows the idle S-tile pool
                    # instead of pot, whose ot is still being normalized)
                    if use_pst:
                        # tail: S tiles and attention accumulators are done;
                        # rotate over all three pools (6 banks) so the casts
                        # never gate the next chain
                        if mc % 3 == 0:
                            ps = pmc.tile([128, 512], F32, tag="qs")
                        elif mc % 3 == 1:
                            pst_tile = pst.tile([128, 1024], F32, tag="st", name="pst_tile")
                            ps = pst_tile[:, 0:512]
                        else:
                            ps = pot.tile([128, 512], F32, tag="ot")
                    elif mc % 2 == 0:
                        ps = pmc.tile([128, 512], F32, tag="qs")
                    else:
                        ps = pot.tile([128, 512], F32, tag="ot")
                    nc.tensor.matmul(
                        ps[:],
                        wprojT[:, ts(mc, 128)],
                        otp[:, ts(qq, 512)],
                        start=True, stop=False,
                    )
                    nc.tensor.matmul(
                        ps[:],
                        wprojT[0:64, 768 + mc * 128 : 768 + (mc + 1) * 128],
                        ots[0:64, ts(qq, 512)],
                        start=False, stop=True,
                    )
                    zb = zpool.tile([128, 512], BF16, tag="zb")
                    # casts_on_scalar: alternate scalar/DVE so the tail casts
                    # drain in parallel on two engines
                    if casts_on_scalar and mc % 2 == 0:
                        nc.scalar.copy(zb[:], ps[:])
                    else:
                        nc.vector.tensor_copy(zb[:], ps[:])
                    nc.sync.dma_start(
                        out=zT_d[ts(mc, 128), ts(qq, 512)], in_=zb[:]
                    )

            for k, (h, qq) in enumerate(groups_order):
                attn_group(h, qq, min(k + 2, len(groups_order) - 1))
                # drain any stragglers for the next group before its PVs
                emit_next_s(3, min(k + 1, len(groups_order) - 1), limit=99)
                if h == 0 and qq > 0:
                    proj(qq - 1, casts_on_scalar=(qq == 3))
            pe_warm(16)
            proj(3, casts_on_scalar=True, use_pst=True)

    nc.compile()
    return nc


@functools.lru_cache(maxsize=1)
def get_nc():
    return build_nc()


def make_in_maps(x, qkv_w, qkv_b, proj_w):
    """Per-core host-side sharding/layout prep."""
    x = np.asarray(x, dtype=np.float32)
    qkv_w = np.asarray(qkv_w, dtype=np.float32)
    qkv_b = np.asarray(qkv_b, dtype=np.float32)
    proj_w = np.asarray(proj_w, dtype=np.float32)

    in_maps = []
    for c in range(8):
        b, g = divmod(c, 4)
        h0, h1, h2 = 3 * g, 3 * g + 1, 3 * g + 2

        def qrows(h):
            return slice(h * HD, (h + 1) * HD)

        def krows(h):
            return slice(D + h * HD, D + (h + 1) * HD)

        def vrows(h):
            return slice(2 * D + h * HD, 2 * D + (h + 1) * HD)

        # qk selection: mc0=[q0|q1] mc1=[k0|k1] mc2=[q2|k2]
        order = [
            qrows(h0), qrows(h1), krows(h0), krows(h1), qrows(h2), krows(h2),
        ]
        wqk = np.concatenate([qkv_w[s] for s in order], axis=0)       # (384, 768)
        # DRAM layout: K block first, then Q, then qk2 (DMA priority order)
        wqkT_host = np.concatenate(
            [
                pmajor(wqk[128:256].T, 128),   # [k0|k1]
                pmajor(wqk[0:128].T, 128),     # [q0|q1]
                pmajor(wqk[256:384].T, 128),   # [q2|k2]
            ],
            axis=1,
        )
        bqk_sel = np.concatenate([qkv_b[s] for s in order], axis=0)   # (384,)
        bcol = np.zeros((128, 4), np.float32)
        for mc in range(3):
            bcol[:, mc] = bqk_sel[mc * 128 : (mc + 1) * 128]
        # natural-V weights: per head 64 cols + one zero col (ones slot)
        wv3 = np.zeros((VW, D), np.float32)
        bv3 = np.zeros(VW, np.float32)
        for i, h in enumerate((h0, h1, h2)):
            wv3[65 * i : 65 * i + 64] = qkv_w[vrows(h)]
            bv3[65 * i : 65 * i + 64] = qkv_b[vrows(h)]
            bv3[65 * i + 64] = 1.0
        wpp = np.concatenate(
            [proj_w[:, ts_np(h0)].T, proj_w[:, ts_np(h1)].T], axis=0
        )  # (128, 768)
        wps = np.concatenate(
            [proj_w[:, ts_np(h2)].T, np.zeros((64, D), np.float32)], axis=0
        )  # (128, 768)
        in_maps.append(
            {
                "xT": np.ascontiguousarray(x[b].reshape(L, D).T).astype(
                    ml_dtypes.bfloat16
                ),
                "wqkT": wqkT_host,
                "wvT3": pmajor(wv3.T, VW),
                "bqk": bcol,
                "bvb": np.broadcast_to(bv3, (128, VW)).copy(),
                "wprojT": np.ascontiguousarray(
                    np.concatenate([wpp, wps], axis=1)
                ).astype(ml_dtypes.bfloat16),
            }
        )
    return in_maps


def ts_np(h):
    return slice(h * HD, (h + 1) * HD)


def pmajor(wT, width):
    """[768, width] contraction-major -> [128, 6*width] partition-major."""
    w3 = np.asarray(wT, np.float32).reshape(NDC, 128, width)
    return np.ascontiguousarray(
        w3.transpose(1, 0, 2).reshape(128, NDC * width)
    ).astype(ml_dtypes.bfloat16)


def assemble_output(results, proj_b):
    proj_b = np.asarray(proj_b, dtype=np.float32)
    out = np.zeros((B, L, D), np.float32)
    for c in range(8):
        b = c // 4
        out[b] += results[c]["zT"].astype(np.float32).T
    out += proj_b[None, None, :]
    return out.reshape(B, T, N, D)


def _install_ntff_hook():
    """The container's antenv stub lacks axon_hooks; recreate it from the
    boot helper so trace=True can profile through libaxon_pjrt."""
    import sys
    import types

    try:
        from antenv.axon_hooks import get_axon_ntff_profile_hook  # noqa: F401

        return
    except ImportError:
        pass
    import antenv
    from trn_agent_boot.trn_boot import _ntff_profile_via_ctypes

    state = {"hook": _ntff_profile_via_ctypes("/opt/axon/libaxon_pjrt.so")}
    mod = types.ModuleType("antenv.axon_hooks")
    mod.set_axon_ntff_profile_hook = lambda h: state.__setitem__("hook", h)
    mod.get_axon_ntff_profile_hook = lambda: state["hook"]
    sys.modules["antenv.axon_hooks"] = mod
    antenv.axon_hooks = mod

    import concourse.bass_utils as bu

    orig_upload = bu.upload_artifacts

    def safe_upload(tmpdir):
        try:
            return orig_upload(tmpdir)
        except Exception:
            return tmpdir

    bu.upload_artifacts = safe_upload


def kernel_with_stats(x, qkv_w, qkv_b, proj_w, proj_b, trace=False):
    from concourse.bass_utils import run_bass_kernel_spmd

    if trace:
        _install_ntff_hook()
    nc = get_nc()
    in_maps = make_in_maps(x, qkv_w, qkv_b, proj_w)
    res = run_bass_kernel_spmd(nc, in_maps, list(range(8)), trace=trace)
    return assemble_output(res.results, proj_b), res


def kernel(x, qkv_w, qkv_b, proj_w, proj_b):
    out, _ = kernel_with_stats(x, qkv_w, qkv_b, proj_w, proj_b)
    return out
